# revision 26
# baseline (speedup 1.0000x reference)
"""Distributed Trainium2 kernel for single-head attention with QKV projections.

Reference computation (B=4, N=4096, D=256, fp32):
    q = x @ Wq_w.T + Wq_b
    k = z @ Wk_w.T + Wk_b
    v = z @ Wv_w.T + Wv_b
    out = softmax(q @ k.T / sqrt(D)) @ v

Sharding: pure data-parallel over (batch, query-half) across 8 cores. Core c
handles batch b = c//2, query rows [h*2048, (h+1)*2048) with h = c%2, and holds
the full z[b] so K/V are recomputed per core (2x duplication of the tiny D*D
projections). No collectives.

Key algebraic fold: q.k = x Wq^T Wk z^T + (bq Wk).z + (x Wq^T bk + bq.bk).
The last group is constant per query row and cancels in softmax, so only
  M  = Wq^T Wk        (folded on host, applied to x on device)
  v2 = (bq Wk)/16     (per-key scalar t3 = v2.z_j; exp(t3) is folded into the
                       V rows rather than biasing the exp of the scores)
survive — the entire K projection disappears; the scores matmul contracts
x M directly against raw z.

Layout: everything is kept "transposed" so no PE transposes are needed:
  - qM[dz, i] comes out of the M-projection with the feature dim on
    partitions, exactly the rhs layout the scores matmul wants; zT (already
    resident) is the lhsT.
  - scores are computed transposed, sT[j, i] (keys on partitions), so the
    exp'd probabilities are directly the lhsT of the PV matmul.
  - scores for two key tiles accumulate into one 2-bank PSUM tile and are
    exp'd by a single [128, 1024] ACTIVATE: the ~300ns/instruction ACT
    overhead was making ScalarE a near-critical engine at one exp per tile.
    This is why exp(t3) moves into the V rows: a biased exp would need a
    distinct per-key-tile [128,1] bias, forcing one ACTIVATE per tile.
  - the V projection emits [Wv^T | v2]-projected tiles; the t3 column is
    gathered across tiles with one strided DVE copy, exp'd in a single
    [128, 32] ACTIVATE, and multiplied into the V rows (DVE tensor_scalar).
    The exp(t3) column itself rides as the 257th V column, so the PV matmul
    still produces the softmax denominator for free.
Compute is bf16 (PE at 1 cycle/row vs 4 for fp32), accumulation fp32 in PSUM.
Softmax skips max-subtraction: scores/sqrt(D) are ~N(0, 0.65) here, bounded
by ~+-4, so exp() is safe in fp32.

Constants are packed host-side into two [128, *] DRAM tensors (Wpack/Bpack):
DMA issue cost on the sequencer is ~5ns per descriptor ~= per partition-row,
so one wide transfer beats many narrow ones. The output is written bf16 and
upcast on the host (bf16 rounding is ~0.2% against a 2% tolerance).

PE clock discipline: the clock ramps 1.2 -> 2.4 GHz after ~3.4us of
continuous matmul activity and RE-THROTTLES after ~3.4us idle. Input-DMA
arrival jitters by microseconds with inter-core HBM contention, so every
potential PE wait in the preamble is kept under that window: a warm-up burst
covers launch -> first xT block, the projection consumes xT in four
separately-DMA'd blocks as they land, and a short dummy-matmul bridge covers
the projection -> z gap. Without these, a slow-DMA run re-throttles
mid-preamble and starts the main loop at half clock (+2-4us).
"""

import numpy as np
import ml_dtypes

B, N, D = 4, 4096, 256
NCORES = 8
S = N // 2          # query rows per core
P = 128             # partitions
QBLK = 512          # scores free-dim block (one PSUM bank)
NJT = N // P        # 32 key tiles
NQB = S // QBLK     # 4 query blocks per core
DC = D // P         # 2 chunks of the feature dim
NWARM = 12          # PE p-state warm-up matmuls (bridge until first input DMA)
VW = D + 1          # vbig tile width: [v(256) | exp(t3)(1)]

BF16 = ml_dtypes.bfloat16

_CACHE = {}


def _build():
    import concourse.mybir as mybir
    import concourse.tile as tile
    from concourse import bacc

    bf16 = mybir.dt.bfloat16
    f32 = mybir.dt.float32
    AF = mybir.ActivationFunctionType
    ALU = mybir.AluOpType

    nc = bacc.Bacc("TRN2", target_bir_lowering=False, debug=False, num_devices=NCORES)

    # Wpack columns: [M chunk0 | M chunk1 | (Wv^T|v2) chunk0 | (Wv^T|v2) chunk1]
    WCOLS = 2 * D + 2 * (D + 1)
    # xT: [128, 2*S] = [chunk0 | chunk1]; zT: [128, 2*N] = [c0h0|c1h0|c0h1|c1h1]
    # (partition-dim chunks packed along the free dim: one DMA of [128, X]
    # costs 128 descriptors regardless of X, so packing halves issue time)
    xT = nc.dram_tensor("xT", [P, DC * S], bf16, kind="ExternalInput").ap()
    zT = nc.dram_tensor("zT", [P, DC * N], bf16, kind="ExternalInput").ap()
    Wpack = nc.dram_tensor("Wpack", [P, WCOLS], bf16, kind="ExternalInput").ap()
    Bpack = nc.dram_tensor("Bpack", [P, D], f32, kind="ExternalInput").ap()
    # bf16 output: the 2e-2 tolerance dwarfs bf16 rounding (~0.2%), and it
    # halves the output bytes on the DMA queue and in the tail flush.
    out = nc.dram_tensor("out", [S, D], bf16, kind="ExternalOutput").ap()

    with tile.TileContext(nc) as tc:
        with (
            tc.tile_pool(name="consts", bufs=1) as cp,
            tc.tile_pool(name="big", bufs=1) as bp,
            tc.tile_pool(name="pblk", bufs=3) as pp,
            tc.tile_pool(name="outp", bufs=8) as op,
            tc.tile_pool(name="psum", bufs=4, space="PSUM") as ps,
        ):
            # ---- PE warm-up (memset on GpSimd: Vector's preamble is slow
            # and would delay the first matmul by >1us) ----
            wrm = cp.tile([P, P + QBLK], bf16, tag="warm", name="warm")
            nc.gpsimd.memset(wrm[:], 0.0)
            wps = ps.tile([P, QBLK], f32, tag="pv", name="warm_ps")
            for _ in range(NWARM):
                nc.tensor.matmul(
                    wps[:], wrm[:, 0:P], wrm[:, P:P + QBLK], start=True, stop=True
                )

            # ---- input / constant DMAs (ordered by first use) ----
            wpk = cp.tile([P, WCOLS], bf16, tag="wpk", name="wpk")
            nc.sync.dma_start(wpk[:], Wpack[:])
            xTp = bp.tile([P, DC * S], bf16, tag="xTp", name="xTp")
            zTp = bp.tile([P, DC * N], bf16, tag="zTp", name="zTp")
            # xT in 4 jb-major blocks so the qM projection tracks the DMA
            # arrival jb-by-jb: a single whole-tensor sem let the PE idle
            # past the ~3.4us HAM window on slow-DMA runs (observed as a
            # mid-preamble re-throttle to 1.2GHz).
            for jb in range(S // QBLK):
                nc.sync.dma_start(
                    xTp[:, jb * 2 * QBLK:(jb + 1) * 2 * QBLK],
                    xT[:, jb * 2 * QBLK:(jb + 1) * 2 * QBLK],
                )
            for h in range(2):
                nc.sync.dma_start(
                    zTp[:, h * N:(h + 1) * N], zT[:, h * N:(h + 1) * N]
                )
            bpk = cp.tile([P, D], f32, tag="bpk", name="bpk")
            nc.sync.dma_start(bpk[:], Bpack[:])

            def xs(c, lo, hi):   # xT chunk c, query cols [lo, hi)
                # jb-major layout: [jb0(c0|c1) | jb1(c0|c1) | ...]
                jb, r = divmod(lo, QBLK)
                assert hi - lo <= QBLK - r
                o = jb * 2 * QBLK + c * QBLK + r
                return xTp[:, o:o + (hi - lo)]

            def zs(c, lo, hi):   # zT chunk c, key cols [lo, hi) (within a half)
                h, r = divmod(lo, N // 2)
                assert hi - lo <= N // 2 - r
                o = h * N + c * (N // 2) + r
                return zTp[:, o:o + (hi - lo)]

            def m_sl(c):   # M lhsT chunk c: [128, 256]
                return wpk[:, c * D:(c + 1) * D]

            def wv_sl(c):  # (Wv^T | v2) rhs chunk c: [128, 257]
                o = 2 * D + c * (D + 1)
                return wpk[:, o:o + D + 1]

            bvb_sb = bpk[:, 0:D]

            # ---- qM projection: (x M)^T[dz, i] over [256, 2048] ----
            # PSUM->SBUF evacuation stays on ScalarE: ACT is idle until the
            # first scores exp (~18us in), while the DVE must chase the V
            # projection to get exp(t3) ready before the first PV matmul.
            qM_sb = []
            for e in range(DC):
                t = bp.tile([P, S], bf16, tag=f"qM{e}", name=f"qM{e}")
                qM_sb.append(t)
            for jb in range(S // QBLK):
                for e in range(DC):
                    acc = ps.tile([P, QBLK], f32, tag="pv", name="proj_ps")
                    for c in range(DC):
                        nc.tensor.matmul(
                            acc[:],
                            m_sl(c)[:, e * P:(e + 1) * P],
                            xs(c, jb * QBLK, (jb + 1) * QBLK),
                            start=(c == 0),
                            stop=(c == DC - 1),
                        )
                    nc.scalar.activation(
                        qM_sb[e][:, jb * QBLK:(jb + 1) * QBLK], acc[:], AF.Copy
                    )

            # ---- p-state bridge ----
            # The gap between the end of the (x-gated) qM projection and the
            # arrival of z's first half varies with inter-core HBM contention;
            # if it exceeds the ~3.4us HAM MID window the PE re-throttles to
            # 1.2GHz and the first microseconds of the main loop run at half
            # clock (observed as +2..4us outliers with a mid-preamble K=4/8
            # HAM event). A short dummy burst keeps the activity monitor busy
            # through the wait; it costs ~0.9us when z is already resident.
            brg = ps.tile([P, 2 * P], f32, tag="sc", name="bridge_ps", bufs=2)
            for _ in range(8):
                nc.tensor.matmul(
                    brg[:], wrm[:, 0:P], wrm[:, P:P + 2 * P], start=True, stop=True
                )

            # ---- v projection: vraw tile t = [v(256) | t3(1)] ----
            vraw = bp.tile([P, NJT * VW], bf16, tag="vraw", name="vraw")
            for t_i in range(NJT):
                acc = ps.tile([P, D + 1], f32, tag="pv", name="v_ps")
                for c in range(DC):
                    nc.tensor.matmul(
                        acc[:],
                        zs(c, t_i * P, (t_i + 1) * P),
                        wv_sl(c),
                        start=(c == 0),
                        stop=(c == DC - 1),
                    )
                o = t_i * VW
                nc.vector.tensor_copy(vraw[:, o:o + VW], acc[:])

            # ---- t3 -> exp(t3), folded into the V rows ----
            # one strided gather of the 32 t3 columns, one [128,32] exp, one
            # strided scatter into vbig's denominator columns, then a
            # per-tile tensor_scalar multiply of the V rows.
            vbig = bp.tile([P, NJT * VW], bf16, tag="vbig", name="vbig")
            t3b = cp.tile([P, NJT], bf16, tag="t3b", name="t3b")
            et3 = cp.tile([P, NJT], f32, tag="et3", name="et3")
            vraw_t3 = vraw[:].rearrange("p (t w) -> p t w", w=VW)[:, :, D:D + 1]
            t3b_3 = t3b[:].rearrange("p (t w) -> p t w", w=1)
            nc.vector.tensor_copy(t3b_3, vraw_t3)
            nc.scalar.activation(et3[:], t3b[:], AF.Exp)
            vbig_t3 = vbig[:].rearrange("p (t w) -> p t w", w=VW)[:, :, D:D + 1]
            et3_3 = et3[:].rearrange("p (t w) -> p t w", w=1)
            nc.vector.tensor_copy(vbig_t3, et3_3)
            for t_i in range(NJT):
                o = t_i * VW
                nc.vector.tensor_scalar_mul(
                    vbig[:, o:o + D], vraw[:, o:o + D], et3[:, t_i:t_i + 1]
                )

            # ---- attention, per query block of 512 ----
            for qb in range(NQB):
                ptb = pp.tile([P, NJT * QBLK], bf16, tag="pT", name="pT")
                for tp in range(NJT // 2):
                    acc = ps.tile([P, 2 * QBLK], f32, tag="sc", name="sc_ps", bufs=2)
                    for half in range(2):
                        t_i = 2 * tp + half
                        for c in range(DC):
                            nc.tensor.matmul(
                                acc[:, half * QBLK:(half + 1) * QBLK],
                                zs(c, t_i * P, (t_i + 1) * P),
                                qM_sb[c][:, qb * QBLK:(qb + 1) * QBLK],
                                start=(c == 0),
                                stop=(c == DC - 1),
                            )
                    # p = exp(scores/16); the query-constant score terms are
                    # dropped (cancel in softmax), the key-constant exp(t3)
                    # factor is already folded into the V rows.
                    nc.scalar.activation(
                        ptb[:, tp * 2 * QBLK:(tp + 1) * 2 * QBLK], acc[:],
                        AF.Exp, scale=1.0 / 16.0,
                    )
                pvs = [
                    ps.tile([P, D + 1], f32, tag="pv", name=f"pv_ps{sq}")
                    for sq in range(QBLK // P)
                ]
                # t-outer so PE consumes exp'd tiles in production order;
                # the last STAG rounds run sq-major so the per-sq
                # accumulations finish staggered and the DVE normalize +
                # output-DMA chains overlap the remaining PV matmuls instead
                # of all queueing after the final one. (STAG=12 was tried to
                # spread the tail DMA flush further but made the schedule
                # unstable: its sq-major section leans on exp readiness.)
                def pt_sl(t_i, sq):
                    o = t_i * QBLK + sq * P
                    return ptb[:, o:o + P]

                def v_sl(t_i):
                    return vbig[:, t_i * VW:t_i * VW + VW]

                STAG = 4
                for t_i in range(NJT - STAG):
                    for sq in range(QBLK // P):
                        nc.tensor.matmul(
                            pvs[sq][:], pt_sl(t_i, sq), v_sl(t_i),
                            start=(t_i == 0), stop=False,
                        )
                for sq in range(QBLK // P):
                    for t_i in range(NJT - STAG, NJT):
                        nc.tensor.matmul(
                            pvs[sq][:], pt_sl(t_i, sq), v_sl(t_i),
                            start=False, stop=(t_i == NJT - 1),
                        )
                for sq in range(QBLK // P):
                    pv = pvs[sq]
                    recip = op.tile([P, 1], f32, tag="recip", name="recip")
                    nc.vector.reciprocal(recip[:], pv[:, D:D + 1])
                    ot = op.tile([P, D], bf16, tag="ot", name="ot")
                    # out = (pv * 1/denom) + bv
                    nc.vector.scalar_tensor_tensor(
                        ot[:], pv[:, 0:D], recip[:], bvb_sb,
                        op0=ALU.mult, op1=ALU.add,
                    )
                    r0 = (qb * (QBLK // P) + sq) * P
                    nc.sync.dma_start(out[r0:r0 + P, :], ot[:])

    nc.compile()
    return nc


def _get_nc():
    if "nc" not in _CACHE:
        _CACHE["nc"] = _build()
    return _CACHE["nc"]


def _prep_in_maps(x, z, Wq_w, Wq_b, Wk_w, Wk_b, Wv_w, Wv_b):
    x = np.asarray(x, np.float32)
    z = np.asarray(z, np.float32)
    Wq = np.asarray(Wq_w, np.float64)
    Wk = np.asarray(Wk_w, np.float64)
    bq = np.asarray(Wq_b, np.float64)

    M = (Wq.T @ Wk).astype(np.float32)           # [dx, dz]
    v2 = ((bq @ Wk) / 16.0).astype(np.float32)   # [dz]
    WvT = np.ascontiguousarray(np.asarray(Wv_w, np.float32).T)  # [dz, e]

    WCOLS = 2 * D + 2 * (D + 1)
    Wpack = np.empty((P, WCOLS), BF16)
    for c in range(DC):
        Wpack[:, c * D:(c + 1) * D] = M[c * P:(c + 1) * P, :].astype(BF16)
        o = 2 * D + c * (D + 1)
        Wpack[:, o:o + D] = WvT[c * P:(c + 1) * P, :].astype(BF16)
        Wpack[:, o + D] = v2[c * P:(c + 1) * P].astype(BF16)
    Bpack = np.ascontiguousarray(
        np.broadcast_to(np.asarray(Wv_b, np.float32), (P, D))
    )

    in_maps = []
    for core in range(NCORES):
        b, h = divmod(core, 2)
        xTc = np.ascontiguousarray(x[b].T[:, h * S:(h + 1) * S]).astype(BF16)
        zTc = np.ascontiguousarray(z[b].T).astype(BF16)
        xTp = np.hstack([
            blk
            for jb in range(S // 512)
            for blk in (xTc[0:P, jb * 512:(jb + 1) * 512],
                        xTc[P:2 * P, jb * 512:(jb + 1) * 512])
        ])
        zTp = np.hstack([
            zTc[0:P, 0:N // 2], zTc[P:2 * P, 0:N // 2],
            zTc[0:P, N // 2:N], zTc[P:2 * P, N // 2:N],
        ])
        in_maps.append({
            "xT": xTp, "zT": zTp,
            "Wpack": Wpack, "Bpack": Bpack,
        })
    return in_maps


def kernel(x, z, Wq_w, Wq_b, Wk_w, Wk_b, Wv_w, Wv_b):
    from concourse.bass_utils import run_bass_kernel_spmd

    in_maps = _prep_in_maps(x, z, Wq_w, Wq_b, Wk_w, Wk_b, Wv_w, Wv_b)
    nc = _get_nc()
    _CACHE["in_maps"] = in_maps
    res = run_bass_kernel_spmd(nc, in_maps, core_ids=list(range(NCORES)))

    full = np.empty((B, N, D), np.float32)
    for core in range(NCORES):
        b, h = divmod(core, 2)
        full[b, h * S:(h + 1) * S, :] = res.results[core]["out"].astype(np.float32)
    return full


# revision 27
# speedup vs baseline: 1.0015x; 1.0015x over previous
"""Distributed Trainium2 kernel for single-head attention with QKV projections.

Reference computation (B=4, N=4096, D=256, fp32):
    q = x @ Wq_w.T + Wq_b
    k = z @ Wk_w.T + Wk_b
    v = z @ Wv_w.T + Wv_b
    out = softmax(q @ k.T / sqrt(D)) @ v

Sharding: pure data-parallel over (batch, query-half) across 8 cores. Core c
handles batch b = c//2, query rows [h*2048, (h+1)*2048) with h = c%2, and holds
the full z[b] so K/V are recomputed per core (2x duplication of the tiny D*D
projections). No collectives.

Key algebraic fold: q.k = x Wq^T Wk z^T + (bq Wk).z + (x Wq^T bk + bq.bk).
The last group is constant per query row and cancels in softmax, so only
  M  = Wq^T Wk        (folded on host, applied to x on device)
  v2 = (bq Wk)/16     (per-key scalar t3 = v2.z_j; exp(t3) is folded into the
                       V rows rather than biasing the exp of the scores)
survive — the entire K projection disappears; the scores matmul contracts
x M directly against raw z.

Layout: everything is kept "transposed" so no PE transposes are needed:
  - qM[dz, i] comes out of the M-projection with the feature dim on
    partitions, exactly the rhs layout the scores matmul wants; zT (already
    resident) is the lhsT.
  - scores are computed transposed, sT[j, i] (keys on partitions), so the
    exp'd probabilities are directly the lhsT of the PV matmul.
  - scores for two key tiles accumulate into one 2-bank PSUM tile and are
    exp'd by a single [128, 1024] ACTIVATE: the ~300ns/instruction ACT
    overhead was making ScalarE a near-critical engine at one exp per tile.
    This is why exp(t3) moves into the V rows: a biased exp would need a
    distinct per-key-tile [128,1] bias, forcing one ACTIVATE per tile.
  - the V projection emits [Wv^T | v2]-projected tiles; the t3 column is
    gathered across tiles with one strided DVE copy, exp'd in a single
    [128, 32] ACTIVATE, and multiplied into the V rows (DVE tensor_scalar).
    The exp(t3) column itself rides as the 257th V column, so the PV matmul
    still produces the softmax denominator for free.
Compute is bf16 (PE at 1 cycle/row vs 4 for fp32), accumulation fp32 in PSUM.
Softmax skips max-subtraction: scores/sqrt(D) are ~N(0, 0.65) here, bounded
by ~+-4, so exp() is safe in fp32.

Constants are packed host-side into two [128, *] DRAM tensors (Wpack/Bpack):
DMA issue cost on the sequencer is ~5ns per descriptor ~= per partition-row,
so one wide transfer beats many narrow ones. The output is written bf16 and
upcast on the host (bf16 rounding is ~0.2% against a 2% tolerance).

PE clock discipline: the clock ramps 1.2 -> 2.4 GHz after ~3.4us of
continuous matmul activity and RE-THROTTLES after ~3.4us idle. Input-DMA
arrival jitters by microseconds with inter-core HBM contention, so every
potential PE wait in the preamble is kept under that window: a warm-up burst
covers launch -> first xT block, the projection consumes xT in four
separately-DMA'd blocks as they land, and a short dummy-matmul bridge covers
the projection -> z gap. Without these, a slow-DMA run re-throttles
mid-preamble and starts the main loop at half clock (+2-4us).
"""

import numpy as np
import ml_dtypes

B, N, D = 4, 4096, 256
NCORES = 8
S = N // 2          # query rows per core
P = 128             # partitions
QBLK = 512          # scores free-dim block (one PSUM bank)
NJT = N // P        # 32 key tiles
NQB = S // QBLK     # 4 query blocks per core
DC = D // P         # 2 chunks of the feature dim
NWARM = 12          # PE p-state warm-up matmuls (bridge until first input DMA)
VW = D + 1          # vbig tile width: [v(256) | exp(t3)(1)]

BF16 = ml_dtypes.bfloat16

_CACHE = {}


def _build():
    import concourse.mybir as mybir
    import concourse.tile as tile
    from concourse import bacc

    bf16 = mybir.dt.bfloat16
    f32 = mybir.dt.float32
    AF = mybir.ActivationFunctionType
    ALU = mybir.AluOpType

    nc = bacc.Bacc("TRN2", target_bir_lowering=False, debug=False, num_devices=NCORES)

    # Wpack columns: [M chunk0 | M chunk1 | (Wv^T|v2) chunk0 | (Wv^T|v2) chunk1]
    WCOLS = 2 * D + 2 * (D + 1)
    # xT: [128, 2*S] = [chunk0 | chunk1]; zT: [128, 2*N] = [c0h0|c1h0|c0h1|c1h1]
    # (partition-dim chunks packed along the free dim: one DMA of [128, X]
    # costs 128 descriptors regardless of X, so packing halves issue time)
    xT = nc.dram_tensor("xT", [P, DC * S], bf16, kind="ExternalInput").ap()
    zT = nc.dram_tensor("zT", [P, DC * N], bf16, kind="ExternalInput").ap()
    Wpack = nc.dram_tensor("Wpack", [P, WCOLS], bf16, kind="ExternalInput").ap()
    Bpack = nc.dram_tensor("Bpack", [P, D], f32, kind="ExternalInput").ap()
    # bf16 output: the 2e-2 tolerance dwarfs bf16 rounding (~0.2%), and it
    # halves the output bytes on the DMA queue and in the tail flush.
    out = nc.dram_tensor("out", [S, D], bf16, kind="ExternalOutput").ap()

    with tile.TileContext(nc) as tc:
        with (
            tc.tile_pool(name="consts", bufs=1) as cp,
            tc.tile_pool(name="big", bufs=1) as bp,
            tc.tile_pool(name="pblk", bufs=3) as pp,
            tc.tile_pool(name="outp", bufs=8) as op,
            tc.tile_pool(name="psum", bufs=4, space="PSUM") as ps,
        ):
            # ---- PE warm-up (memset on GpSimd: Vector's preamble is slow
            # and would delay the first matmul by >1us) ----
            wrm = cp.tile([P, P + QBLK], bf16, tag="warm", name="warm")
            nc.gpsimd.memset(wrm[:], 0.0)
            wps = ps.tile([P, QBLK], f32, tag="pv", name="warm_ps")
            for _ in range(NWARM):
                nc.tensor.matmul(
                    wps[:], wrm[:, 0:P], wrm[:, P:P + QBLK], start=True, stop=True
                )

            # ---- input / constant DMAs (ordered by first use) ----
            wpk = cp.tile([P, WCOLS], bf16, tag="wpk", name="wpk")
            nc.sync.dma_start(wpk[:], Wpack[:])
            xTp = bp.tile([P, DC * S], bf16, tag="xTp", name="xTp")
            zTp = bp.tile([P, DC * N], bf16, tag="zTp", name="zTp")
            # xT in 4 jb-major blocks so the qM projection tracks the DMA
            # arrival jb-by-jb: a single whole-tensor sem let the PE idle
            # past the ~3.4us HAM window on slow-DMA runs (observed as a
            # mid-preamble re-throttle to 1.2GHz).
            for jb in range(S // QBLK):
                nc.sync.dma_start(
                    xTp[:, jb * 2 * QBLK:(jb + 1) * 2 * QBLK],
                    xT[:, jb * 2 * QBLK:(jb + 1) * 2 * QBLK],
                )
            # z in 8 transfers: per half, the two chunks' first quarters
            # land first (they cover key tiles 0-7 of the half), so the V
            # projection and scores start ~1.4us earlier and track the
            # stream. (Safe now that the HAM bridge covers idle gaps; an
            # earlier attempt without the bridge sampled an unlucky
            # re-throttle run and looked like a regression.)
            ZH = N // 2   # 2048: one chunk-block within a half
            ZQ = ZH // 2  # 1024: quarter transfer
            for h in range(2):
                for q in range(2):
                    for c in range(2):
                        lo = h * N + c * ZH + q * ZQ
                        nc.sync.dma_start(zTp[:, lo:lo + ZQ], zT[:, lo:lo + ZQ])
            bpk = cp.tile([P, D], f32, tag="bpk", name="bpk")
            nc.sync.dma_start(bpk[:], Bpack[:])

            def xs(c, lo, hi):   # xT chunk c, query cols [lo, hi)
                # jb-major layout: [jb0(c0|c1) | jb1(c0|c1) | ...]
                jb, r = divmod(lo, QBLK)
                assert hi - lo <= QBLK - r
                o = jb * 2 * QBLK + c * QBLK + r
                return xTp[:, o:o + (hi - lo)]

            def zs(c, lo, hi):   # zT chunk c, key cols [lo, hi) (within a half)
                h, r = divmod(lo, N // 2)
                assert hi - lo <= N // 2 - r
                o = h * N + c * (N // 2) + r
                return zTp[:, o:o + (hi - lo)]

            def m_sl(c):   # M lhsT chunk c: [128, 256]
                return wpk[:, c * D:(c + 1) * D]

            def wv_sl(c):  # (Wv^T | v2) rhs chunk c: [128, 257]
                o = 2 * D + c * (D + 1)
                return wpk[:, o:o + D + 1]

            bvb_sb = bpk[:, 0:D]

            # ---- qM projection: (x M)^T[dz, i] over [256, 2048] ----
            # PSUM->SBUF evacuation stays on ScalarE: ACT is idle until the
            # first scores exp (~18us in), while the DVE must chase the V
            # projection to get exp(t3) ready before the first PV matmul.
            qM_sb = []
            for e in range(DC):
                t = bp.tile([P, S], bf16, tag=f"qM{e}", name=f"qM{e}")
                qM_sb.append(t)
            for jb in range(S // QBLK):
                for e in range(DC):
                    acc = ps.tile([P, QBLK], f32, tag="pv", name="proj_ps")
                    for c in range(DC):
                        nc.tensor.matmul(
                            acc[:],
                            m_sl(c)[:, e * P:(e + 1) * P],
                            xs(c, jb * QBLK, (jb + 1) * QBLK),
                            start=(c == 0),
                            stop=(c == DC - 1),
                        )
                    nc.scalar.activation(
                        qM_sb[e][:, jb * QBLK:(jb + 1) * QBLK], acc[:], AF.Copy
                    )

            # ---- p-state bridge ----
            # The gap between the end of the (x-gated) qM projection and the
            # arrival of z's first half varies with inter-core HBM contention;
            # if it exceeds the ~3.4us HAM MID window the PE re-throttles to
            # 1.2GHz and the first microseconds of the main loop run at half
            # clock (observed as +2..4us outliers with a mid-preamble K=4/8
            # HAM event). A short dummy burst keeps the activity monitor busy
            # through the wait; it costs ~0.9us when z is already resident.
            brg = ps.tile([P, 2 * P], f32, tag="sc", name="bridge_ps", bufs=2)
            for _ in range(8):
                nc.tensor.matmul(
                    brg[:], wrm[:, 0:P], wrm[:, P:P + 2 * P], start=True, stop=True
                )

            # ---- v projection: vraw tile t = [v(256) | t3(1)] ----
            vraw = bp.tile([P, NJT * VW], bf16, tag="vraw", name="vraw")
            for t_i in range(NJT):
                acc = ps.tile([P, D + 1], f32, tag="pv", name="v_ps")
                for c in range(DC):
                    nc.tensor.matmul(
                        acc[:],
                        zs(c, t_i * P, (t_i + 1) * P),
                        wv_sl(c),
                        start=(c == 0),
                        stop=(c == DC - 1),
                    )
                o = t_i * VW
                nc.vector.tensor_copy(vraw[:, o:o + VW], acc[:])

            # ---- t3 -> exp(t3), folded into the V rows ----
            # one strided gather of the 32 t3 columns, one [128,32] exp, one
            # strided scatter into vbig's denominator columns, then a
            # per-tile tensor_scalar multiply of the V rows.
            vbig = bp.tile([P, NJT * VW], bf16, tag="vbig", name="vbig")
            t3b = cp.tile([P, NJT], bf16, tag="t3b", name="t3b")
            et3 = cp.tile([P, NJT], f32, tag="et3", name="et3")
            vraw_t3 = vraw[:].rearrange("p (t w) -> p t w", w=VW)[:, :, D:D + 1]
            t3b_3 = t3b[:].rearrange("p (t w) -> p t w", w=1)
            nc.vector.tensor_copy(t3b_3, vraw_t3)
            nc.scalar.activation(et3[:], t3b[:], AF.Exp)
            vbig_t3 = vbig[:].rearrange("p (t w) -> p t w", w=VW)[:, :, D:D + 1]
            et3_3 = et3[:].rearrange("p (t w) -> p t w", w=1)
            nc.vector.tensor_copy(vbig_t3, et3_3)
            for t_i in range(NJT):
                o = t_i * VW
                nc.vector.tensor_scalar_mul(
                    vbig[:, o:o + D], vraw[:, o:o + D], et3[:, t_i:t_i + 1]
                )

            # ---- attention, per query block of 512 ----
            for qb in range(NQB):
                ptb = pp.tile([P, NJT * QBLK], bf16, tag="pT", name="pT")
                for tp in range(NJT // 2):
                    acc = ps.tile([P, 2 * QBLK], f32, tag="sc", name="sc_ps", bufs=2)
                    for half in range(2):
                        t_i = 2 * tp + half
                        for c in range(DC):
                            nc.tensor.matmul(
                                acc[:, half * QBLK:(half + 1) * QBLK],
                                zs(c, t_i * P, (t_i + 1) * P),
                                qM_sb[c][:, qb * QBLK:(qb + 1) * QBLK],
                                start=(c == 0),
                                stop=(c == DC - 1),
                            )
                    # p = exp(scores/16); the query-constant score terms are
                    # dropped (cancel in softmax), the key-constant exp(t3)
                    # factor is already folded into the V rows.
                    nc.scalar.activation(
                        ptb[:, tp * 2 * QBLK:(tp + 1) * 2 * QBLK], acc[:],
                        AF.Exp, scale=1.0 / 16.0,
                    )
                pvs = [
                    ps.tile([P, D + 1], f32, tag="pv", name=f"pv_ps{sq}")
                    for sq in range(QBLK // P)
                ]
                # t-outer so PE consumes exp'd tiles in production order;
                # the last STAG rounds run sq-major so the per-sq
                # accumulations finish staggered and the DVE normalize +
                # output-DMA chains overlap the remaining PV matmuls instead
                # of all queueing after the final one. (STAG=12 was tried to
                # spread the tail DMA flush further but made the schedule
                # unstable: its sq-major section leans on exp readiness.)
                def pt_sl(t_i, sq):
                    o = t_i * QBLK + sq * P
                    return ptb[:, o:o + P]

                def v_sl(t_i):
                    return vbig[:, t_i * VW:t_i * VW + VW]

                STAG = 4
                for t_i in range(NJT - STAG):
                    for sq in range(QBLK // P):
                        nc.tensor.matmul(
                            pvs[sq][:], pt_sl(t_i, sq), v_sl(t_i),
                            start=(t_i == 0), stop=False,
                        )
                for sq in range(QBLK // P):
                    for t_i in range(NJT - STAG, NJT):
                        nc.tensor.matmul(
                            pvs[sq][:], pt_sl(t_i, sq), v_sl(t_i),
                            start=False, stop=(t_i == NJT - 1),
                        )
                for sq in range(QBLK // P):
                    pv = pvs[sq]
                    recip = op.tile([P, 1], f32, tag="recip", name="recip")
                    nc.vector.reciprocal(recip[:], pv[:, D:D + 1])
                    ot = op.tile([P, D], bf16, tag="ot", name="ot")
                    # out = (pv * 1/denom) + bv
                    nc.vector.scalar_tensor_tensor(
                        ot[:], pv[:, 0:D], recip[:], bvb_sb,
                        op0=ALU.mult, op1=ALU.add,
                    )
                    r0 = (qb * (QBLK // P) + sq) * P
                    nc.sync.dma_start(out[r0:r0 + P, :], ot[:])

    nc.compile()
    return nc


def _get_nc():
    if "nc" not in _CACHE:
        _CACHE["nc"] = _build()
    return _CACHE["nc"]


def _prep_in_maps(x, z, Wq_w, Wq_b, Wk_w, Wk_b, Wv_w, Wv_b):
    x = np.asarray(x, np.float32)
    z = np.asarray(z, np.float32)
    Wq = np.asarray(Wq_w, np.float64)
    Wk = np.asarray(Wk_w, np.float64)
    bq = np.asarray(Wq_b, np.float64)

    M = (Wq.T @ Wk).astype(np.float32)           # [dx, dz]
    v2 = ((bq @ Wk) / 16.0).astype(np.float32)   # [dz]
    WvT = np.ascontiguousarray(np.asarray(Wv_w, np.float32).T)  # [dz, e]

    WCOLS = 2 * D + 2 * (D + 1)
    Wpack = np.empty((P, WCOLS), BF16)
    for c in range(DC):
        Wpack[:, c * D:(c + 1) * D] = M[c * P:(c + 1) * P, :].astype(BF16)
        o = 2 * D + c * (D + 1)
        Wpack[:, o:o + D] = WvT[c * P:(c + 1) * P, :].astype(BF16)
        Wpack[:, o + D] = v2[c * P:(c + 1) * P].astype(BF16)
    Bpack = np.ascontiguousarray(
        np.broadcast_to(np.asarray(Wv_b, np.float32), (P, D))
    )

    in_maps = []
    for core in range(NCORES):
        b, h = divmod(core, 2)
        xTc = np.ascontiguousarray(x[b].T[:, h * S:(h + 1) * S]).astype(BF16)
        zTc = np.ascontiguousarray(z[b].T).astype(BF16)
        xTp = np.hstack([
            blk
            for jb in range(S // 512)
            for blk in (xTc[0:P, jb * 512:(jb + 1) * 512],
                        xTc[P:2 * P, jb * 512:(jb + 1) * 512])
        ])
        zTp = np.hstack([
            zTc[0:P, 0:N // 2], zTc[P:2 * P, 0:N // 2],
            zTc[0:P, N // 2:N], zTc[P:2 * P, N // 2:N],
        ])
        in_maps.append({
            "xT": xTp, "zT": zTp,
            "Wpack": Wpack, "Bpack": Bpack,
        })
    return in_maps


def kernel(x, z, Wq_w, Wq_b, Wk_w, Wk_b, Wv_w, Wv_b):
    from concourse.bass_utils import run_bass_kernel_spmd

    in_maps = _prep_in_maps(x, z, Wq_w, Wq_b, Wk_w, Wk_b, Wv_w, Wv_b)
    nc = _get_nc()
    _CACHE["in_maps"] = in_maps
    res = run_bass_kernel_spmd(nc, in_maps, core_ids=list(range(NCORES)))

    full = np.empty((B, N, D), np.float32)
    for core in range(NCORES):
        b, h = divmod(core, 2)
        full[b, h * S:(h + 1) * S, :] = res.results[core]["out"].astype(np.float32)
    return full
